# revision 19
# baseline (speedup 1.0000x reference)
"""Trainium2 Bass kernel for the Jastrow-factor nn.Module.

Math (per walker w):
  EN: r_en[w,e,n] = |x_we - nuc_n|
      J_en   = sum_{e,n} -q_n * r/(1+softplus(b_en_n)*r)
      J_ennn = s_en * sum_e MLP8(r_en[w,e,:]**2)        (8->32->32->1, silu)
  EE: r_ee[w,p] over 496 unordered pairs p=(i,j)
      J_ee   = sum_p a_p * r/(1+softplus(b_ee)*r)
      J_eenn = s_ee * sum_p MLP1(r_ee[w,p])             (1->32->32->1, silu)
  out[w] = J_en + J_ennn + J_ee + J_eenn

Distribution: pure data parallel, 2048 walkers per core on 4 cores.
(4 cores, not 8: the metric is warm host wall time, which one execute
round trip through the PJRT tunnel dominates; fan-out to 8 cores costs
~5ms more per call in terminal-side launch/sync than 4, while the extra
device work per core stays far below the round-trip floor. 1/2/4-core
launch costs are equal, and 4-core halves per-core SBUF pressure vs 2.)

This problem is dispatch/transfer-bound, not compute-bound: device math is
<1ms while each host->device round trip through the PJRT tunnel costs tens
of ms. So the kernel ships only two small arrays per call — the raw
electron coordinates (a zero-copy view of r_electrons) and a 24KB packed
weight blob — and builds every derived operand (identity, block-diagonal
MLP weights, the EN distance matrix, selection structures) on device.
The jitted executable is built once and cached across calls.

Device program per core (2048 walkers, processed as two 1024-walker
halves so the PSUM accumulator rows fit in 8 banks):
  EN: coords+|x|^2 are PE-transposed to [feature, walker] layout; one
      dense matmul against a sparse nuclei matrix produces |x|^2-2x.nuc,
      and the ACT sqrt adds |nuc|^2 via its per-partition bias to give
      r_en. The 8->32->32->1 MLP runs as block-diagonal matmuls over
      (electron,nucleus) partition blocks; L3 + the classical
      charge-weighted term accumulate into one PSUM row.
  EE: pair distances via 31 diagonal-offset subtractions in
      [128 walker-partitions, free] layout, one big ACT sqrt, PE
      transposes into 4 tiles [124 pairs, 1024 walkers]. The MLP uses
      per-group row-selection weight matrices (K=124, built on device)
      so every matmul operand sits at partition base 0; L2 is
      block-diagonal; L3 and the classical term accumulate into one
      PSUM row.
"""

import numpy as np

N_CORES = 8
N_W, N_E, N_NUC, D_H = 8192, 32, 8, 32
WC = N_W // N_CORES          # walkers per core
NT = WC // 128               # walker tiles per core (8)
P_PAIRS = N_E * (N_E - 1) // 2   # 496
NB = 4                       # rT pair tiles, 124 pairs each
PB = P_PAIRS // NB           # 124
NSEL = PB // 4               # 31 m-steps per pair tile
BW = 48                      # blob width (f32 columns)


def _pair_list():
    ps = []
    for d in range(1, N_E):
        for e in range(N_E - d):
            ps.append((e, e + d))
    return ps


_PAIRS = _pair_list()
assert len(_PAIRS) == P_PAIRS


def _softplus(x):
    return np.log1p(np.exp(-np.abs(x))) + np.maximum(x, 0.0)


# ----------------------------------------------------------------------------
# device program
# ----------------------------------------------------------------------------

_CACHE = {}


def _build_program():
    from contextlib import ExitStack

    import concourse.bacc as bacc
    import concourse.bass as bass
    import concourse.tile as tile
    from concourse import mybir
    from concourse.masks import make_identity

    f32 = mybir.dt.float32
    f16 = mybir.dt.float16
    AF = mybir.ActivationFunctionType
    ALU = mybir.AluOpType

    nc = bacc.Bacc()

    d_xin = nc.declare_dram_parameter("xin", [NT, 128, 96], f16, isOutput=False)
    d_blob = nc.declare_dram_parameter("blob", [128, BW], f32, isOutput=False)
    d_out = nc.declare_dram_parameter("out", [1, WC], f32, isOutput=True)

    MM = nc.tensor.matmul

    with ExitStack() as top:
        tc = top.enter_context(tile.TileContext(nc))
        const = top.enter_context(tc.tile_pool(name="const", bufs=1))
        work = top.enter_context(tc.tile_pool(name="work", bufs=1))

        blob = const.tile([128, BW], f32, name="blob", tag="blob")
        nc.gpsimd.dma_start(out=blob[:], in_=d_blob[:])

        # blob column map (see _shared_blob)
        b1en = blob[:, 32:33]
        b2en = blob[:, 33:34]
        wenl3 = blob[:, 34:35]
        qneg = blob[:, 35:36]
        bensp = blob[:, 36:37]
        qnb = blob[:, 37:38]
        b1ee = blob[:, 38:39]
        b2ee = blob[:, 39:40]
        weel3 = blob[:, 40:41]
        beesp = blob[:, 41:42]
        cconst = blob[0:1, 43:44]

        # ------------------------------------------------------------------
        # on-device constant construction
        # ------------------------------------------------------------------
        ident = const.tile([128, 128], f32, name="ident", tag="ident")
        make_identity(nc, ident[:])

        # W1B4R: 4x vstack of blockdiag4(W1_en [8,32])
        w1b4r = const.tile([128, 128], f32, name="w1b4r", tag="w1b4r")
        nc.vector.memset(w1b4r[:], 0.0)
        for s in range(4):
            for el in range(4):
                nc.gpsimd.dma_start(
                    out=w1b4r[32 * s + 8 * el : 32 * s + 8 * el + 8,
                              32 * el : 32 * el + 32],
                    in_=d_blob[0:8, 0:32],
                )

        # blockdiag4(W2_en), blockdiag4(W2_ee)
        wenl2 = const.tile([128, 128], f32, name="wenl2", tag="wenl2")
        weel2 = const.tile([128, 128], f32, name="weel2", tag="weel2")
        nc.vector.memset(wenl2[:], 0.0)
        nc.vector.memset(weel2[:], 0.0)
        for el in range(4):
            nc.vector.tensor_copy(
                wenl2[32 * el : 32 * el + 32, 32 * el : 32 * el + 32],
                blob[32:64, 0:32],
            )
            nc.vector.tensor_copy(
                weel2[32 * el : 32 * el + 32, 32 * el : 32 * el + 32],
                blob[64:96, 0:32],
            )

        # EE L1 selection stack: weesel[4m+j, m, 32j:32j+32] = W1_ee[0,:]
        # (engine ops need 32-aligned partition bases; DMA does not, so all
        # scattered-partition constant builds below go through the DMA queue)
        weesel = const.tile([PB, NSEL, 128], f32, name="weesel", tag="weesel")
        nc.vector.memset(weesel[:], 0.0)
        for p in range(PB):
            nc.gpsimd.dma_start(
                out=weesel[p : p + 1, p // 4, 32 * (p % 4) : 32 * (p % 4) + 32],
                in_=d_blob[99:100, 0:32],
            )

        # EN distance matrices: wend[h][48h+3e'+cc, 8e'+n] = -2 nuc[n,cc],
        # wend[h][96+16h+e', 8e'+n] = 1   (e' = e - 16h)
        wend = [
            const.tile([128, 128], f32, name=f"wend{h}", tag=f"wend{h}")
            for h in range(2)
        ]
        for h in range(2):
            nc.vector.memset(wend[h][:], 0.0)
            for e2 in range(16):
                nc.gpsimd.dma_start(
                    out=wend[h][48 * h + 3 * e2 : 48 * h + 3 * e2 + 3,
                                8 * e2 : 8 * e2 + 8],
                    in_=d_blob[96:99, 0:8],
                )
                nc.gpsimd.dma_start(
                    out=wend[h][96 + 16 * h + e2 : 96 + 16 * h + e2 + 1,
                                8 * e2 : 8 * e2 + 8],
                    in_=d_blob[100:101, 0:8],
                )

        # ------------------------------------------------------------------
        # load coords; xaug[:, t, 0:96] = coords, [:, t, 96+e] = |x_we|^2
        # ------------------------------------------------------------------
        xaug = work.tile([128, NT, 128], f32, name="xaug")
        with tc.tile_pool(name="x16p", bufs=1) as x16p:
            x16 = x16p.tile([128, NT, 96], f16, tag="x16")
            for t in range(NT):
                nc.gpsimd.dma_start(out=x16[:, t, :], in_=d_xin[t])
            nc.vector.tensor_copy(xaug[:, :, 0:96], x16[:])
        with tc.tile_pool(name="sqp", bufs=1) as sqp:
            sq = sqp.tile([128, NT, 96], f32, tag="sq")
            nc.vector.tensor_mul(sq[:], xaug[:, :, 0:96], xaug[:, :, 0:96])
            sq3 = sq[:].rearrange("p t (e c) -> p c t e", c=3)
            nc.vector.tensor_add(xaug[:, :, 96:128], sq3[:, 0], sq3[:, 1])
            nc.vector.tensor_add(
                xaug[:, :, 96:128], xaug[:, :, 96:128], sq3[:, 2]
            )

        # ------------------------------------------------------------------
        # EE distances in walker-partition layout
        # r2wp[p, t, col] ; col = pair index by diagonal order, padded to 512
        # ------------------------------------------------------------------
        r2wp = work.tile([128, NT, 512], f32)
        nc.vector.memset(r2wp[:], 0.0)
        with tc.tile_pool(name="dpool", bufs=2) as dpool:
            off = 0
            for d in range(1, N_E):
                L = N_E - d
                dd = dpool.tile([128, NT, 96], f32, tag="dd")
                sqd = dpool.tile([128, NT, 96], f32, tag="sqd")
                nc.vector.tensor_sub(
                    dd[:, :, : 3 * L], xaug[:, :, : 3 * L],
                    xaug[:, :, 3 * d : 3 * d + 3 * L],
                )
                nc.vector.tensor_mul(
                    sqd[:, :, : 3 * L], dd[:, :, : 3 * L], dd[:, :, : 3 * L]
                )
                s3 = sqd[:, :, : 3 * L].rearrange("p t (e c) -> p c t e", c=3)
                nc.vector.tensor_add(r2wp[:, :, off : off + L], s3[:, 0], s3[:, 1])
                nc.vector.tensor_add(
                    r2wp[:, :, off : off + L], r2wp[:, :, off : off + L], s3[:, 2]
                )
                off += L
            assert off == P_PAIRS

        rwp = r2wp
        nc.scalar.sqrt(rwp[:], r2wp[:])

        # ------------------------------------------------------------------
        # EN: transpose xaug -> xT [feature, walker]
        # ------------------------------------------------------------------
        xT = work.tile([128, WC], f32, name="xT")
        with tc.tile_pool(name="xtps", bufs=2, space=bass.MemorySpace.PSUM) as xtps:
            for t in range(NT):
                pt = xtps.tile([128, 128], f32, tag="pt")
                nc.tensor.transpose(pt[:], xaug[:, t, :], ident[:])
                nc.vector.tensor_copy(xT[:, 128 * t : 128 * t + 128], pt[:])

        # r_en in [p=(e',n), (h, w)] layout; renT = sqrt(wend.T@xT + |nuc|^2)
        renT = work.tile([128, 2, WC], f32, name="renT")
        r2sb = work.tile([128, 2, WC], f32, name="r2sb")
        with tc.tile_pool(name="enps", bufs=2, space=bass.MemorySpace.PSUM) as enps:
            for h in range(2):
                for hw in range(2):
                    ps = enps.tile([128, 512], f32, tag="ps")
                    MM(ps[:], wend[h][:], xT[:, 512 * hw : 512 * hw + 512],
                       start=True, stop=True)
                    nc.scalar.activation(
                        renT[:, h, 512 * hw : 512 * hw + 512], ps[:],
                        AF.Sqrt, bias=qnb,
                    )
        nc.vector.tensor_mul(r2sb[:], renT[:], renT[:])

        # EN classical: ten = r / (1 + softplus(b_en)*r)
        uen = work.tile([128, 2, WC], f32, name="uen")
        nc.vector.tensor_scalar(
            uen[:], renT[:], bensp, 1.0, op0=ALU.mult, op1=ALU.add
        )
        nc.vector.reciprocal_approx_fast(out=uen[:], in_=uen[:])
        ten = renT
        nc.vector.tensor_mul(ten[:], renT[:], uen[:])

        # ------------------------------------------------------------------
        # EN MLP + classical accumulation -> jen_sb[1, 1024]
        # ------------------------------------------------------------------
        jen_sb = work.tile([1, WC], f32, name="jen_sb")
        with (
            tc.tile_pool(name="enp1", bufs=2, space=bass.MemorySpace.PSUM) as enp1,
            tc.tile_pool(name="enp2", bufs=1, space=bass.MemorySpace.PSUM) as enp2,
            tc.tile_pool(name="enj", bufs=1, space=bass.MemorySpace.PSUM) as enj,
            tc.tile_pool(name="enh", bufs=2) as enh,
        ):
            jen = enj.tile([1, WC], f32)
            for hw in range(2):
                for h in range(2):
                    MM(jen[0:1, 512 * hw : 512 * hw + 512], qneg,
                       ten[:, h, 512 * hw : 512 * hw + 512],
                       start=(h == 0), stop=False, skip_group_check=True)
            for h in range(2):
                for s in range(4):
                    ps1 = enp1.tile([128, 2, 512], f32, tag="ps1")
                    for hw in range(2):
                        MM(ps1[:, hw, :], w1b4r[32 * s : 32 * s + 32, :],
                           r2sb[32 * s : 32 * s + 32, h, 512 * hw : 512 * hw + 512],
                           start=True, stop=True, tile_position=(32 * s, 0))
                    h1 = enh.tile([128, 2, 512], f32, tag="h1")
                    nc.scalar.activation(h1[:], ps1[:], AF.Silu, bias=b1en)
                    ps2 = enp2.tile([128, 2, 512], f32, tag="ps2")
                    for hw in range(2):
                        MM(ps2[:, hw, :], wenl2[:], h1[:, hw, :],
                           start=True, stop=True)
                    h2 = enh.tile([128, 2, 512], f32, tag="h2")
                    nc.scalar.activation(h2[:], ps2[:], AF.Silu, bias=b2en)
                    last = h == 1 and s == 3
                    for hw in range(2):
                        MM(jen[0:1, 512 * hw : 512 * hw + 512], wenl3,
                           h2[:, hw, :],
                           start=False, stop=last, skip_group_check=True)
            nc.vector.tensor_copy(jen_sb[:], jen[:])

        # ------------------------------------------------------------------
        # EE transposes: rwp -> rT[b] [124 pairs, 1024 walkers]
        # ------------------------------------------------------------------
        rT = [work.tile([PB, WC], f32, tag=f"rT{b}", name=f"rT{b}") for b in range(NB)]
        with tc.tile_pool(name="ptps", bufs=3, space=bass.MemorySpace.PSUM) as ptps:
            for t in range(NT):
                for b in range(NB):
                    pt = ptps.tile([PB, 128], f32, tag="pt")
                    nc.tensor.transpose(
                        pt[:], rwp[:, t, PB * b : PB * b + PB], ident[:]
                    )
                    nc.vector.tensor_copy(rT[b][:, 128 * t : 128 * t + 128], pt[:])

        # ------------------------------------------------------------------
        # EE classical + MLP, accumulating into jee[1, 1024] (PSUM)
        # ------------------------------------------------------------------
        with (
            tc.tile_pool(name="jeeps", bufs=1, space=bass.MemorySpace.PSUM) as jeeps,
            tc.tile_pool(name="eecls", bufs=2) as eecls,
        ):
            jee = jeeps.tile([1, WC], f32)
            for b in range(NB):
                u = eecls.tile([PB, WC], f32, tag="u")
                nc.vector.tensor_scalar(
                    u[:], rT[b][:], beesp[0:PB], 1.0, op0=ALU.mult, op1=ALU.add
                )
                nc.vector.reciprocal_approx_fast(out=u[:], in_=u[:])
                t_ee = eecls.tile([PB, WC], f32, tag="t")
                nc.vector.tensor_mul(t_ee[:], rT[b][:], u[:])
                for hw in range(2):
                    MM(
                        jee[0:1, 512 * hw : 512 * hw + 512],
                        blob[0:PB, 44 + b : 45 + b],
                        t_ee[:, 512 * hw : 512 * hw + 512],
                        start=(b == 0), stop=False, skip_group_check=True,
                    )

            with (
                tc.tile_pool(
                    name="eeps1", bufs=2, space=bass.MemorySpace.PSUM
                ) as eeps1,
                tc.tile_pool(
                    name="eeps2", bufs=1, space=bass.MemorySpace.PSUM
                ) as eeps2,
                tc.tile_pool(name="eeh", bufs=2) as eeh,
            ):
                for q in range(PB):
                    b, m = divmod(q, NSEL)
                    ps1 = eeps1.tile([128, 2, 512], f32, tag="ps1")
                    for hw in range(2):
                        MM(ps1[:, hw, :], weesel[:, m, :],
                           rT[b][:, 512 * hw : 512 * hw + 512],
                           start=True, stop=True)
                    h1 = eeh.tile([128, 2, 512], f32, tag="h1")
                    nc.scalar.activation(h1[:], ps1[:], AF.Silu, bias=b1ee)
                    ps2 = eeps2.tile([128, 2, 512], f32, tag="ps2")
                    for hw in range(2):
                        MM(ps2[:, hw, :], weel2[:], h1[:, hw, :],
                           start=True, stop=True)
                    h2 = eeh.tile([128, 2, 512], f32, tag="h2")
                    nc.scalar.activation(h2[:], ps2[:], AF.Silu, bias=b2ee)
                    last = q == PB - 1
                    for hw in range(2):
                        MM(jee[0:1, 512 * hw : 512 * hw + 512], weel3,
                           h2[:, hw, :],
                           start=False, stop=last, skip_group_check=True)

                # final: out = (jee + C) + jen
                out_sb = work.tile([1, WC], f32)
                nc.vector.scalar_tensor_tensor(
                    out=out_sb[:],
                    in0=jee[:],
                    scalar=cconst,
                    in1=jen_sb[:],
                    op0=ALU.add,
                    op1=ALU.add,
                )
                nc.gpsimd.dma_start(out=d_out[:], in_=out_sb[:])

    nc.finalize()
    return nc


def _get_program():
    if "nc" not in _CACHE:
        _CACHE["nc"] = _build_program()
    return _CACHE["nc"]


# ----------------------------------------------------------------------------
# cached PJRT runner (one jitted executable reused across calls)
# ----------------------------------------------------------------------------


def _get_runner():
    if "runner" in _CACHE:
        return _CACHE["runner"]

    import jax
    from jax.experimental.shard_map import shard_map
    from jax.sharding import Mesh, PartitionSpec

    from concourse import bass2jax, mybir

    nc = _get_program()
    bass2jax.install_neuronx_cc_hook()

    partition_name = nc.partition_id_tensor.name if nc.partition_id_tensor else None
    in_names, out_names, out_avals, zero_shapes = [], [], [], []
    for alloc in nc.m.functions[0].allocations:
        if not isinstance(alloc, mybir.MemoryLocationSet):
            continue
        name = alloc.memorylocations[0].name
        if alloc.kind == "ExternalInput":
            if name != partition_name:
                in_names.append(name)
        elif alloc.kind == "ExternalOutput":
            shape = tuple(alloc.tensor_shape)
            dtype = mybir.dt.np(alloc.dtype)
            out_avals.append(jax.core.ShapedArray(shape, dtype))
            zero_shapes.append((shape, dtype))
            out_names.append(name)
    n_params = len(in_names)
    n_outs = len(out_avals)
    in_names_all = in_names + out_names + (
        [partition_name] if partition_name else []
    )
    donate = tuple(range(n_params, n_params + n_outs))

    def _body(*args):
        operands = list(args)
        if partition_name is not None:
            operands.append(bass2jax.partition_id_tensor())
        outs = bass2jax._bass_exec_p.bind(
            *operands,
            out_avals=tuple(out_avals),
            in_names=tuple(in_names_all),
            out_names=tuple(out_names),
            lowering_input_output_aliases=(),
            sim_require_finite=True,
            sim_require_nnan=True,
            nc=nc,
        )
        return tuple(outs)

    devices = jax.devices()[:N_CORES]
    mesh = Mesh(np.asarray(devices), ("core",))
    in_specs = (PartitionSpec("core"),) * (n_params + n_outs)
    out_specs = (PartitionSpec("core"),) * len(out_names)
    fn = jax.jit(
        shard_map(
            _body, mesh=mesh, in_specs=in_specs, out_specs=out_specs,
            check_rep=False,
        ),
        donate_argnums=donate,
        keep_unused=True,
    )

    def runner(global_in_map):
        args = [global_in_map[name] for name in in_names]
        # donated output buffers; the kernel writes every element, so no
        # zero-init is needed
        args += [
            np.empty((N_CORES * s[0], *s[1:]), d) for s, d in zero_shapes
        ]
        outs = fn(*args)
        return {name: np.asarray(outs[i]) for i, name in enumerate(out_names)}

    # Warm the executable and the device transport: the first couple of
    # executions pay one-time costs (NEFF load, transfer-pool setup), so
    # run two dummy executions now to make subsequent calls steady-state.
    dummy = {
        "xin": np.zeros((N_CORES * NT, 128, 96), np.float16),
        "blob": np.zeros((N_CORES * 128, BW), np.float32),
    }
    for _ in range(2):
        runner(dummy)

    _CACHE["runner"] = runner
    return runner


# ----------------------------------------------------------------------------
# host-side input prep
# ----------------------------------------------------------------------------


def _shared_blob(r_nuclei, charges, spin_mask_parallel, b_en, b_ee,
                 W1_en, b1_en, W2_en, b2_en, W3_en, b3_en,
                 W1_ee, b1_ee, W2_ee, b2_ee, W3_ee, b3_ee,
                 scale_en, scale_ee):
    f = np.float32
    nuc = np.asarray(r_nuclei, f)
    q = np.asarray(charges, f)
    sm = np.asarray(spin_mask_parallel)
    s_en = float(np.asarray(scale_en))
    s_ee = float(np.asarray(scale_ee))

    blob = np.zeros((128, BW), f)
    blob[0:8, 0:32] = np.asarray(W1_en, f)
    blob[32:64, 0:32] = np.asarray(W2_en, f)
    blob[64:96, 0:32] = np.asarray(W2_ee, f)
    blob[96:99, 0:8] = -2.0 * nuc.T
    blob[99, 0:32] = np.asarray(W1_ee, f).reshape(32)
    blob[100, 0:8] = 1.0

    blob[:, 32] = np.tile(np.asarray(b1_en, f).reshape(32), 4)
    blob[:, 33] = np.tile(np.asarray(b2_en, f).reshape(32), 4)
    blob[:, 34] = np.tile(s_en * np.asarray(W3_en, f).reshape(32), 4)
    blob[:, 35] = np.tile(-q, 16)
    blob[:, 36] = np.tile(_softplus(np.asarray(b_en, f)).reshape(8), 16)
    blob[:, 37] = np.tile((nuc ** 2).sum(-1), 16)
    blob[:, 38] = np.tile(np.asarray(b1_ee, f).reshape(32), 4)
    blob[:, 39] = np.tile(np.asarray(b2_ee, f).reshape(32), 4)
    blob[:, 40] = np.tile(s_ee * np.asarray(W3_ee, f).reshape(32), 4)
    blob[:, 41] = float(_softplus(np.asarray(b_ee, f).reshape(1))[0])

    a_all = np.where(
        sm[tuple(np.array(_PAIRS).T)], f(0.25), f(0.5)
    ).astype(f)
    blob[0:PB, 44:48] = a_all.reshape(NB, PB).T

    blob[0, 43] = N_E * s_en * float(np.asarray(b3_en).reshape(-1)[0]) + \
        P_PAIRS * s_ee * float(np.asarray(b3_ee).reshape(-1)[0])
    return blob


def _run(inputs, trace=False):
    from types import SimpleNamespace

    runner = _get_runner()
    blob = _shared_blob(
        inputs["r_nuclei"], inputs["charges"], inputs["spin_mask_parallel"],
        inputs["b_en"], inputs["b_ee"],
        inputs["W1_en"], inputs["b1_en"], inputs["W2_en"], inputs["b2_en"],
        inputs["W3_en"], inputs["b3_en"],
        inputs["W1_ee"], inputs["b1_ee"], inputs["W2_ee"], inputs["b2_ee"],
        inputs["W3_ee"], inputs["b3_ee"],
        inputs["scale_en"], inputs["scale_ee"],
    )
    r_el = np.asarray(inputs["r_electrons"], np.float32)
    # persistent staging buffers: a fresh 1.6MB allocation per call costs
    # ~2ms in page faults, which is measurable next to the ~55ms call
    if "xin_buf" not in _CACHE:
        _CACHE["xin_buf"] = np.empty((N_CORES * NT, 128, 96), np.float16)
        _CACHE["blob_buf"] = np.empty((N_CORES, 128, BW), np.float32)
    xin = _CACHE["xin_buf"]
    np.copyto(xin, r_el.reshape(N_CORES * NT, 128, 96), casting="same_kind")
    blob_b = _CACHE["blob_buf"]
    blob_b[:] = blob
    blob_g = blob_b.reshape(N_CORES * 128, BW)
    res = runner({"xin": xin, "blob": blob_g})
    out = res["out"].reshape(-1).astype(np.float32, copy=False)
    return out, SimpleNamespace(exec_time_ns=None, results=None)


def kernel(**inputs):
    out, _ = _run(inputs, trace=False)
    return out
